# revision 17
# baseline (speedup 1.0000x reference)
"""BankModulatedConv Trainium2 kernel, v3: kh-granular bank streaming.

Per sample b of B=8, one NeuronCore per sample:
  w = softmax(bank_request[b]); kern = sum_f w[f]*bank_weight[f]
  kern *= (1+style); kern *= rsqrt(sum_{i,khw} kern^2 + 1e-8)
  y[b] = conv2d(x[b], kern, SAME)

v3 layout: the three PE-mixed blocks (0,0),(0,1),(1,1) ship as kh-row
tiles [i(128), (f16, kw3, o128)] so a mix mini (16 accumulated diag
matmuls, N=384) completes per kh as soon as its 1.57MB tile lands; conv
kh-passes chase the minis, putting the PE to work from ~14us. The MAC
block (1,0) keeps fq tiles for the VectorE scalar_tensor_tensor chain.
All dense data is bf16 (f32 PSUM accumulation); y ships bf16.
"""
import sys

if "/opt/trn_rl_repo" not in sys.path:
    sys.path.insert(0, "/opt/trn_rl_repo")

import numpy as np
import concourse.bacc as bacc
import concourse.mybir as mybir
import concourse.tile as tile
from concourse.alu_op_type import AluOpType
from concourse.bass_utils import run_bass_kernel_spmd

dt = mybir.dt
AF = mybir.ActivationFunctionType

B, F, D, KK, H, W = 8, 16, 256, 3, 64, 64
HW = H * W
KHW = KK * KK
IC = D // 128
OC = D // 128
FQ = 4
FPQ = F // FQ
OCK = 128 * KHW       # 1152: per-(oc,ic) kernel cols, (kh, kw, o) major
KROW = F * KK * 128   # 6144: kh-tile cols (f, kw, o)
MROW = FPQ * OCK      # 4608: mac fq-tile cols (f4, khw, o)
PW = W + 2
PH_ = H + 2
NS = 8
SROWS = H // NS
SN = SROWS * W        # 512
XA_ROWS = 34
GRP = 4

PE_BLOCKS = ((0, 0), (0, 1), (1, 1))   # diag-matmul mixed
MAC_BLOCK = (1, 0)                      # DVE MAC chain

_COMPILED = None


def _build(num_devices=B):
    nc = bacc.Bacc("TRN2", target_bir_lowering=False, debug=False,
                   num_devices=num_devices)

    x_d = nc.dram_tensor("x", [D, PH_ * PW], dt.bfloat16, kind="ExternalInput").ap()
    bankpe_d = nc.dram_tensor("bankpe", [len(PE_BLOCKS) * KK * 128, KROW],
                              dt.bfloat16, kind="ExternalInput").ap()
    bankmac_d = nc.dram_tensor("bankmac", [FQ * 128, MROW], dt.bfloat16,
                               kind="ExternalInput").ap()
    consts_d = nc.dram_tensor("consts", [128, 257], dt.bfloat16,
                              kind="ExternalInput").ap()
    breq_d = nc.dram_tensor("breq", [1, F], dt.float32, kind="ExternalInput").ap()
    sty_d = nc.dram_tensor("sty", [1, D], dt.float32, kind="ExternalInput").ap()
    y_d = nc.dram_tensor("y", [D, HW], dt.bfloat16, kind="ExternalOutput").ap()

    f32, f32r, bf16 = dt.float32, dt.float32r, dt.bfloat16

    with tile.TileContext(nc) as tc:
        with (
            tc.tile_pool(name="setup", bufs=1) as setup,
            tc.tile_pool(name="xp", bufs=1) as xp,
            tc.tile_pool(name="bankp", bufs=2) as bankp,
            tc.tile_pool(name="bankmacp", bufs=4) as bankmacp,
            tc.tile_pool(name="kern", bufs=1) as kernp,
            tc.tile_pool(name="yout", bufs=4) as youtp,
            tc.tile_pool(name="mixps", bufs=1, space="PSUM") as mixps,
            tc.tile_pool(name="convps", bufs=5, space="PSUM") as convps,
            tc.tile_pool(name="normps", bufs=1, space="PSUM") as normps,
        ):
            # control DMAs ride the Vector/GpSimd descriptor queues so the
            # first bank tile is the FIRST Sync-queue issue (each DIRECT2D
            # costs ~650ns of serial descriptor writing per sequencer)
            breq = setup.tile([1, F], dt.float32)
            nc.scalar.dma_start(breq[:], breq_d[:])
            styrow = setup.tile([1, D], dt.float32)
            nc.scalar.dma_start(styrow[:], sty_d[:])
            consts = setup.tile([128, 257], bf16)
            nc.gpsimd.dma_start(consts[:], consts_d[:])

            khts = {}
            mts = {}

            def issue_kh_dma(bi, oc, ic, kh):
                bt = bankp.tile([128, KROW], bf16, tag=f"bank{oc}{ic}",
                                name=f"bank{oc}{ic}k{kh}")
                row0 = (bi * KK + kh) * 128
                nc.sync.dma_start(bt[:], bankpe_d[row0:row0 + 128, :])
                khts[(oc, ic, kh)] = bt

            # ---------- DMA issue order == arrival order ----------
            xpads = []
            for ic in range(IC):
                xpad = xp.tile([128, PH_ * PW], bf16, tag=f"xpad{ic}",
                               name=f"xpad{ic}")
                xpads.append(xpad)
            issue_kh_dma(0, 0, 0, 0)   # first mix mini's tile leads
            for ic in range(IC):
                nc.sync.dma_start(xpads[ic][:, 0:XA_ROWS * PW],
                                  x_d[ic * 128:(ic + 1) * 128, 0:XA_ROWS * PW])
            issue_kh_dma(0, 0, 0, 1)
            issue_kh_dma(0, 0, 0, 2)
            for kh in range(KK):
                issue_kh_dma(1, 0, 1, kh)
            for ic in range(IC):
                nc.sync.dma_start(xpads[ic][:, XA_ROWS * PW:],
                                  x_d[ic * 128:(ic + 1) * 128, XA_ROWS * PW:])
            for fq in range(FQ):
                mt = bankmacp.tile([128, MROW], bf16, tag="bankmac",
                                   name=f"bankmac{fq}")
                nc.sync.dma_start(mt[:], bankmac_d[fq * 128:(fq + 1) * 128, :])
                mts[fq] = mt
            for kh in range(KK):
                issue_kh_dma(2, 1, 1, kh)

            # ---------- setup ----------
            ident = consts[:, 0:128]
            onescol = consts[:, 128:129]
            onesrow_b = consts[0:1, 129:257]
            ones11_b = consts[0:1, 129:130]

            # unnormalized softmax: the 1/sum(exp) factor cancels in the L2
            # demodulation (kern/||kern|| is scale-invariant; the 1e-8 eps is
            # negligible against ||kern||^2 ~ O(1e2)), so mix with raw exp
            ex = setup.tile([1, F], f32)
            nc.scalar.activation(ex[:], breq[:], AF.Exp, bias=0.0, scale=1.0)
            wrow_b = setup.tile([1, F], bf16)
            with nc.allow_low_precision(reason="broadcast only"):
                nc.vector.tensor_copy(wrow_b[:], ex[:])
            auxps = normps.tile([128, 512], f32, tag="aux")
            wbps = auxps[:, 0:F]
            nc.tensor.matmul(wbps[:], onesrow_b[:], wrow_b[:], start=True, stop=True)
            wbc = setup.tile([128, F], f32)
            nc.vector.tensor_copy(wbc[:], wbps[:])

            diags = []
            with nc.allow_low_precision(reason="bf16 diag weights; mix accumulates f32"):
                for f in range(F):
                    dg = setup.tile([128, 128], bf16, tag=f"diag{f}")
                    nc.vector.tensor_scalar(out=dg[:], in0=ident[:],
                                            scalar1=wbc[:, f:f + 1],
                                            scalar2=None, op0=AluOpType.mult)
                    diags.append(dg)

            sty1 = setup.tile([1, D], f32)
            nc.scalar.activation(sty1[:], styrow[:], AF.Copy, bias=1.0, scale=1.0)
            sty1b = setup.tile([1, D], bf16)
            with nc.allow_low_precision(reason="style factors"):
                nc.vector.tensor_copy(sty1b[:], sty1[:])
            styps = auxps[:, 16:16 + IC]
            stycols = []
            for ic in range(IC):
                nc.tensor.matmul(styps[:, ic:ic + 1],
                                 sty1b[0:1, ic * 128:(ic + 1) * 128],
                                 ones11_b, start=True, stop=True)
                sc = setup.tile([128, 1], f32, tag=f"sty{ic}")
                nc.scalar.activation(sc[:], styps[:, ic:ic + 1], AF.Copy,
                                     bias=0.0, scale=1.0)
                stycols.append(sc)

            # dummy Sqrt: force the sqrt table (which also holds Copy) to
            # load now, while ScalarE is idle, not at first demod use
            warm = setup.tile([1, 1], f32)
            nc.scalar.activation(warm[:], ex[:, 0:1], AF.Sqrt, bias=1.0, scale=1.0)

            sty2cols = []
            for ic in range(IC):
                s2 = setup.tile([128, 1], f32, tag=f"sty2{ic}", name=f"sty2{ic}")
                nc.scalar.activation(s2[:], stycols[ic][:], AF.Square,
                                     bias=0.0, scale=1.0)
                s2r = setup.tile([128, 1], f32r, tag=f"sty2r{ic}", name=f"sty2r{ic}")
                nc.vector.tensor_copy(s2r[:], s2[:])
                sty2cols.append(s2r)

            ones_r = setup.tile([128, 1], f32r)
            nc.vector.tensor_copy(ones_r[:], onescol)
            ones12 = setup.tile([1, 2], f32)
            nc.vector.memset(ones12[:], 1.0)

            km = {}
            normcols = {}

            def kt_alloc(oc, ic):
                kt = kernp.tile([128, OCK], bf16, tag=f"kern{oc}{ic}",
                                name=f"kt{oc}{ic}")
                km[(ic, oc)] = kt
                return kt

            KWO = KK * 128  # 384 cols per kh mini

            scrs = {}

            def mix_mini(oc, ic, kh):
                # 16 accumulated diag matmuls over one kh tile, then the
                # style-scaled PSUM->SBUF copy (ScalarE) into the kernel tile.
                # The demod square reads the PSUM directly (pre-style; style^2
                # rides the i-reduction matmul), so the norm path never waits
                # on the style copy.
                bt = khts[(oc, ic, kh)]
                ps = mixps.tile([128, KWO], f32, tag=f"mix{kh % 2}",
                                name=f"ps{oc}{ic}{kh}")
                for f in range(F):
                    nc.tensor.matmul(ps[:], diags[f][:],
                                     bt[:, f * KWO:(f + 1) * KWO],
                                     start=(f == 0), stop=(f == F - 1))
                kt = km[(ic, oc)]
                with nc.allow_low_precision(reason="conv runs bf16"):
                    for kw in range(KK):
                        dst = kt[:, kh * KWO + kw * 128:kh * KWO + (kw + 1) * 128]
                        src_ = ps[:, kw * 128:(kw + 1) * 128]
                        if kw == 1:
                            nc.vector.tensor_scalar(
                                out=dst, in0=src_, scalar1=stycols[ic][:],
                                scalar2=None, op0=AluOpType.mult)
                        else:
                            nc.scalar.activation(dst, src_, AF.Copy, bias=0.0,
                                                 scale=stycols[ic][:])
                if (oc, ic) not in scrs:
                    scrs[(oc, ic)] = kernp.tile([128, OCK], f32,
                                                tag=f"sq{oc}{ic}",
                                                name=f"sq{oc}{ic}")
                nc.scalar.activation(scrs[(oc, ic)][:, kh * KWO:(kh + 1) * KWO],
                                     ps[:], AF.Square, bias=0.0, scale=1.0)

            def demod_reduce_pre(oc, ic):
                redk = kernp.tile([128, 128], f32r, tag=f"redk{oc}{ic}",
                                  name=f"redk{oc}{ic}")
                with nc.allow_low_precision(reason="f32r is 4-byte"):
                    nc.vector.tensor_reduce(
                        redk[:],
                        scrs[(oc, ic)][:, :].rearrange("p (r o) -> p o r", r=KHW),
                        axis=mybir.AxisListType.X, op=AluOpType.add)
                return redk

            def mix_mac(oc, ic):
                kt = kt_alloc(oc, ic)
                acc0 = kernp.tile([128, OCK], f32, tag="macacc0", name="macacc0")
                acc1 = kernp.tile([128, OCK], f32, tag="macacc1", name="macacc1")
                accs = (acc0, acc1)
                with nc.allow_low_precision(reason="bf16 in, f32 acc"):
                    nc.vector.tensor_scalar(
                        out=accs[0][:], in0=mts[0][:, 0:OCK],
                        scalar1=wbc[:, 0:1], scalar2=None, op0=AluOpType.mult)
                    for f in range(1, F):
                        mt = mts[f // FPQ]
                        fo = (f % FPQ) * OCK
                        nc.vector.scalar_tensor_tensor(
                            out=accs[f % 2][:], in0=mt[:, fo:fo + OCK],
                            scalar=wbc[:, f:f + 1], in1=accs[(f + 1) % 2][:],
                            op0=AluOpType.mult, op1=AluOpType.add)
                    nc.vector.tensor_scalar(
                        out=kt[:], in0=accs[(F - 1) % 2][:],
                        scalar1=stycols[ic][:], scalar2=None, op0=AluOpType.mult)

            def demod_dve(oc, ic):
                kt = km[(ic, oc)]
                scr = kernp.tile([128, OCK], f32r, tag="sqscratch", name=f"scr{oc}{ic}")
                nc.vector.tensor_mul(scr[:], kt[:], kt[:])
                redk = kernp.tile([128, 128], f32r, tag=f"redk{oc}{ic}",
                                  name=f"redk{oc}{ic}")
                with nc.allow_low_precision(reason="f32r is 4-byte"):
                    nc.vector.tensor_reduce(
                        redk[:], scr[:, :].rearrange("p (r o) -> p o r", r=KHW),
                        axis=mybir.AxisListType.X, op=AluOpType.add)
                return redk

            def demod_pe(npsum, redk, lhs, first, last):
                nc.tensor.matmul(npsum[:], lhs[:], redk[:],
                                 start=first, stop=last)

            def norm_final(oc, npsum):
                # sqrt straight off PSUM on ScalarE, one short reciprocal on
                # VectorE. The reference's 1e-8 eps is dropped: ||kern||^2 is
                # a sum of 2304 squares, O(1e2), so the eps shifts the result
                # by ~1e-9 relative -- far below the bf16 noise floor.
                nsq = setup.tile([1, 128], f32, tag=f"nsq{oc}", name=f"nsq{oc}")
                nc.scalar.activation(nsq[:], npsum[:], AF.Sqrt, bias=0.0, scale=1.0)
                nrec = setup.tile([1, 128], f32, tag=f"nrec{oc}", name=f"nrec{oc}")
                nc.vector.reciprocal(nrec[:], nsq[:])
                ntr = auxps[:, 20 + 2 * oc:22 + 2 * oc]
                nc.tensor.matmul(ntr[:], nrec[:], ones12[:], start=True, stop=True)
                ncol = setup.tile([128, 1], f32, tag=f"ncol{oc}", name=f"ncol{oc}")
                nc.scalar.activation(ncol[:], ntr[:, 0:1], AF.Copy, bias=0.0, scale=1.0)
                normcols[oc] = ncol

            cps_group = {}

            def conv_alloc(oc, g):
                tiles = []
                for s in range(g * GRP, (g + 1) * GRP):
                    tiles.append(convps.tile([128, SN], f32, tag="conv",
                                             name=f"c{oc}{s}"))
                cps_group[(oc, g)] = tiles

            def conv_pass(oc, g, ic, kh_list, first_ic, last_ic):
                tiles = cps_group[(oc, g)]
                kt = km[(ic, oc)]
                xv = xpads[ic][:, :].rearrange("p (r c) -> p r c", c=PW)
                for kh in kh_list:
                    for kw in range(KK):
                        kslice = kt[:, (kh * KK + kw) * 128:(kh * KK + kw + 1) * 128]
                        st = first_ic and kh == 0 and kw == 0
                        sp = last_ic and kh == KK - 1 and kw == KK - 1
                        for si, s in enumerate(range(g * GRP, (g + 1) * GRP)):
                            r0 = s * SROWS
                            rhs = xv[:, r0 + kh: r0 + kh + SROWS, kw:kw + W]
                            nc.tensor.matmul(tiles[si][:], kslice, rhs,
                                             start=st, stop=sp)

            def conv_out(oc, g, split=False):
                with nc.allow_low_precision(reason="y ships bf16"):
                    for si, s in enumerate(range(g * GRP, (g + 1) * GRP)):
                        r0 = s * SROWS
                        yt = youtp.tile([128, SN], bf16, tag="y", name=f"y{oc}{s}")
                        if split and si % 2 == 1:
                            nc.vector.tensor_scalar(
                                out=yt[:], in0=cps_group[(oc, g)][si][:],
                                scalar1=normcols[oc][:], scalar2=None,
                                op0=AluOpType.mult)
                        else:
                            nc.scalar.activation(yt[:], cps_group[(oc, g)][si][:],
                                                 AF.Copy, bias=0.0,
                                                 scale=normcols[oc][:])
                        dma_eng = nc.sync if (split and si % 2 == 1) else nc.gpsimd
                        dma_eng.dma_start(
                            y_d[oc * 128:(oc + 1) * 128, r0 * W:(r0 + SROWS) * W],
                            yt[:])

            # ============== static PE schedule ==============
            kt00 = kt_alloc(0, 0)  # noqa: F841
            kt01 = kt_alloc(0, 1)  # noqa: F841
            conv_alloc(0, 0)

            mix_mini(0, 0, 0)
            conv_pass(0, 0, 0, [0], True, False)
            mix_mini(0, 0, 1)
            conv_pass(0, 0, 0, [1], False, False)
            mix_mini(0, 0, 2)
            conv_pass(0, 0, 0, [2], False, False)

            mix_mini(0, 1, 0)
            redk00 = demod_reduce_pre(0, 0)
            conv_pass(0, 0, 1, [0], False, False)
            mix_mini(0, 1, 1)
            npsum0 = auxps[0:1, 128:256]
            demod_pe(npsum0, redk00, sty2cols[0], True, False)
            conv_pass(0, 0, 1, [1], False, False)
            mix_mini(0, 1, 2)
            redk01 = demod_reduce_pre(0, 1)
            conv_pass(0, 0, 1, [2], False, True)
            demod_pe(npsum0, redk01, sty2cols[1], False, True)
            norm_final(0, npsum0)
            conv_out(0, 0)

            conv_alloc(0, 1)
            conv_pass(0, 1, 0, [0, 1, 2], True, False)
            mix_mac(1, 0)
            redk10 = demod_dve(1, 0)
            conv_pass(0, 1, 1, [0, 1, 2], False, True)
            conv_out(0, 1)

            kt11 = kt_alloc(1, 1)  # noqa: F841
            npsum1 = auxps[0:1, 256:384]
            conv_alloc(1, 0)
            mix_mini(1, 1, 0)
            mix_mini(1, 1, 1)
            conv_pass(1, 0, 0, [0], True, False)
            mix_mini(1, 1, 2)
            redk11 = demod_reduce_pre(1, 1)
            demod_pe(npsum1, redk10, ones_r, True, False)
            conv_pass(1, 0, 0, [1], False, False)
            demod_pe(npsum1, redk11, sty2cols[1], False, True)
            conv_pass(1, 0, 0, [2], False, False)
            norm_final(1, npsum1)
            conv_pass(1, 0, 1, [0, 1, 2], False, True)
            conv_out(1, 0)

            conv_alloc(1, 1)
            conv_pass(1, 1, 0, [0, 1, 2], True, False)
            conv_pass(1, 1, 1, [0, 1, 2], False, True)
            conv_out(1, 1, split=True)

    nc.compile()
    return nc


def _get_compiled():
    global _COMPILED
    if _COMPILED is None:
        _COMPILED = _build()
    return _COMPILED


def _make_in_maps(x, bank_request, style, bank_weight):
    bf16_np = mybir.dt.np(mybir.dt.bfloat16)
    W5 = bank_weight.astype(np.float32).reshape(F, OC, 128, IC, 128, KK, KK)
    #                                   dims:   (f, oc, o, ic, i, kh, kw)
    pe_parts = []
    for oc, ic in PE_BLOCKS:
        sub = W5[:, oc, :, ic, :, :, :]          # (f, o, i, kh, kw)
        # rows (kh, i) x cols (f, kw, o)
        pe_parts.append(sub.transpose(3, 2, 0, 4, 1).reshape(KK * 128, KROW))
    bank_pe = np.ascontiguousarray(np.concatenate(pe_parts, axis=0)).astype(bf16_np)

    moc, mic = MAC_BLOCK
    sub = W5[:, moc, :, mic, :, :, :].reshape(FQ, FPQ, 128, 128, KK, KK)
    #                                   dims: (fq, fl, o, i, kh, kw)
    bank_mac = np.ascontiguousarray(
        sub.transpose(0, 3, 1, 4, 5, 2).reshape(FQ * 128, MROW)).astype(bf16_np)

    consts = np.zeros((128, 257), dtype=np.float32)
    consts[:, 0:128] = np.eye(128, dtype=np.float32)
    consts[:, 128] = 1.0
    consts[0, 129:257] = 1.0
    consts = np.ascontiguousarray(consts).astype(bf16_np)

    xpad = np.zeros((B, D, PH_, PW), dtype=np.float32)
    xpad[:, :, 1:1 + H, 1:1 + W] = x.astype(np.float32).reshape(B, D, H, W)
    xpad = xpad.astype(bf16_np)
    maps = []
    for b in range(B):
        maps.append({
            "x": np.ascontiguousarray(xpad[b].reshape(D, PH_ * PW)),
            "bankpe": bank_pe,
            "bankmac": bank_mac,
            "consts": consts,
            "breq": np.ascontiguousarray(
                bank_request[b].astype(np.float32).reshape(1, F)),
            "sty": np.ascontiguousarray(style[b].astype(np.float32).reshape(1, D)),
        })
    return maps


def run(inputs, trace=False, **trace_kwargs):
    nc = _get_compiled()
    in_maps = _make_in_maps(inputs["x"], inputs["bank_request"],
                            inputs["style"], inputs["bank_weight"])
    last_exc = None
    for _ in range(3):
        try:
            res = run_bass_kernel_spmd(nc, in_maps, core_ids=list(range(B)),
                                       trace=trace, **trace_kwargs)
            y = np.stack([res.results[b]["y"].astype(np.float32).reshape(D, H, W)
                          for b in range(B)], axis=0)
            return y, res
        except Exception as e:  # noqa: BLE001
            last_exc = e
    raise last_exc


def kernel(x, bank_request, style, bank_weight):
    y, _ = run({"x": np.asarray(x), "bank_request": np.asarray(bank_request),
                "style": np.asarray(style), "bank_weight": np.asarray(bank_weight)})
    return y
